# revision 14
# baseline (speedup 1.0000x reference)
"""Trainium2 Bass kernel for the CrossAttention problem.

Math (per batch b, reference semantics):
  mol_q  = Wq  @ mol + bq          (256 = 8 heads x 32)
  mol_v  = Wmv @ mol + bmv
  prot_k = P @ Wpk.T + bpk         (P = prot_features[b], [S, 128])
  prot_v = P @ Wpv.T + bpv
  raw[s,h]   = <prot_k[s,h,:], mol_q[h,:]> = P[s,:] @ kvT[:,h] + kc[h]
  m2p_w      = softmax_s(scale * raw)
  p2m_w      = softmax_h(raw)
  out_mol    = Wmo @ (sum_s m2p_w * prot_v) + bmo + mol
  out_prot   = p2m_w-weighted mol_v heads @ Wpo.T + bpo + P
  out_avg    = mean_h m2p_w

All projection weights are folded algebraically into tiny per-head
matrices (G/H/M', computed on device in a prologue) so that the big
tensor P is read from HBM exactly once and the big output written once:

  kvT[:,h] = G_h.T @ mol + g_h,  G_h = Wq_h.T @ Wpk_h, g_h = Wpk_h.T bq_h
  The kc[h] shift cancels in every m2p quantity (softmax over s), and for
  the p2m path exp(raw+kc) = exp(raw)*e^kc with e^kc folded into the U
  rows and the denominator column:
  A_h = sum_s exp(scale*raw)[s,h] * [P[s,:] | 1]   (psum accum)
  out_mol = sum_h M'_h.T @ (A_h/A_h[128]) + (Wmo@bpv+bmo) + mol
  out_prot[s,:] = (sum_h exp(raw)[s,h] * ek_h*[U'_h|1]) / den[s] + P[s,:]
    U'_h = H_h.T @ mol + WpoT_h.T @ bmv_h + bpo,  ek_h = e^{kc[h]}

Sharding: pure data parallel, 4 batches per core x 8 cores.

Hardware constraints honored here:
 - matmul operands must start at partition 0/32/64 -> per-head 32-row
   weight slices live in [64,128] tiles (head pairs); p2m exp tiles are
   transposed per 128-seq tile so lhsT sits at offset 0.
 - matmul start=True clears has_written for the WHOLE psum bank ->
   every matmul is single-shot except the deliberate accumulations
   (A in its own bank, attmol per tile in its own bank, bias_vec in its
   own prologue bank); the shared s1 scratch bank only ever holds
   single-shot matmul outputs.
"""

import sys

if "/opt/trn_rl_repo" not in sys.path:
    sys.path.insert(0, "/opt/trn_rl_repo")

import numpy as np

import concourse.bacc as bacc
import concourse.tile as tile
from concourse import mybir

F32 = mybir.dt.float32
EXP = mybir.ActivationFunctionType.Exp
MULT = mybir.AluOpType.mult
ADD = mybir.AluOpType.add

N_CORES = 8
B_FULL, S_FULL = 32, 4096
H, D, C, A = 8, 32, 128, 256
SCALE = 1.0 / float(np.sqrt(D))
GT = 8  # seq tiles per group


def build_program(b_core, s, debug=False):
    """One SPMD NeuronCore program handling b_core batches of seq len s."""
    t_tiles = s // 128
    n_grp = t_tiles // GT
    assert t_tiles % GT == 0

    nc = bacc.Bacc(None, target_bir_lowering=False)

    io = {}
    io["prot"] = nc.dram_tensor("prot", [b_core, s, C], F32, kind="ExternalInput")
    io["molT"] = nc.dram_tensor("molT", [C, b_core], F32, kind="ExternalInput")
    for nm in ("wq", "wmv", "wpk", "wpv"):
        io[nm] = nc.dram_tensor(nm, [A, C], F32, kind="ExternalInput")
    io["wmo"] = nc.dram_tensor("wmo", [C, A], F32, kind="ExternalInput")
    io["wpo"] = nc.dram_tensor("wpo", [C, A], F32, kind="ExternalInput")
    # col k*4+j = bias_k[j*64:(j+1)*64], k in (bq, bmv, bpk, bpv)
    io["baug"] = nc.dram_tensor("baug", [64, 16], F32, kind="ExternalInput")
    io["bcol"] = nc.dram_tensor("bcol", [C, 2], F32, kind="ExternalInput")  # bpo,bmo
    io["brow"] = nc.dram_tensor("brow", [1, 2 * A], F32, kind="ExternalInput")  # bq|bpk
    io["ident"] = nc.dram_tensor("ident", [C, C], F32, kind="ExternalInput")
    io["onescol"] = nc.dram_tensor("onescol", [C, 1], F32, kind="ExternalInput")
    io["onesrow"] = nc.dram_tensor("onesrow", [1, C], F32, kind="ExternalInput")

    io["out_prot"] = nc.dram_tensor("out_prot", [b_core, s, C], F32,
                                    kind="ExternalOutput")
    io["out_mol"] = nc.dram_tensor("out_mol", [b_core, C], F32,
                                   kind="ExternalOutput")
    io["out_avg"] = nc.dram_tensor("out_avg", [b_core, s], F32,
                                   kind="ExternalOutput")
    io["debug"] = debug
    if debug:
        io["dbg_sc"] = nc.dram_tensor("dbg_sc", [C, GT * H], F32,
                                      kind="ExternalOutput")
        io["dbg_eb"] = nc.dram_tensor("dbg_eb", [C, 2 * GT * H], F32,
                                      kind="ExternalOutput")
        io["dbg_kvt"] = nc.dram_tensor("dbg_kvt", [C, H], F32,
                                       kind="ExternalOutput")
        io["dbg_un"] = nc.dram_tensor("dbg_un", [H, C + 1], F32,
                                      kind="ExternalOutput")
        io["dbg_a"] = nc.dram_tensor("dbg_a", [H, C + 1], F32,
                                     kind="ExternalOutput")

    with tile.TileContext(nc) as tc:
        _emit(nc, tc, b_core, s, t_tiles, n_grp, io)
    nc.compile()
    return nc


def _emit(nc, tc, b_core, s, t_tiles, n_grp, io):
    from contextlib import ExitStack

    prot = io["prot"]
    out_prot, out_mol, out_avg = io["out_prot"], io["out_mol"], io["out_avg"]

    def hof(h):  # head -> (pair tile index, row slice at offset 0/32)
        return h // 2, slice((h % 2) * D, (h % 2) * D + D)

    ctx = ExitStack()
    with ctx:
        consts = ctx.enter_context(tc.tile_pool(name="consts", bufs=1))

        # ---- constants / weights into SBUF ----
        ident = consts.tile([C, C], F32)
        nc.sync.dma_start(out=ident, in_=io["ident"][:, :])
        onescol = consts.tile([C, 1], F32)
        nc.sync.dma_start(out=onescol, in_=io["onescol"][:, :])
        onesrow = consts.tile([1, C], F32)
        nc.sync.dma_start(out=onesrow, in_=io["onesrow"][:, :])
        baug = consts.tile([64, 16], F32)
        nc.sync.dma_start(out=baug, in_=io["baug"][:, :])
        bcol = consts.tile([C, 2], F32)
        nc.sync.dma_start(out=bcol, in_=io["bcol"][:, :])
        brow = consts.tile([1, 2 * A], F32)
        nc.sync.dma_start(out=brow, in_=io["brow"][:, :])
        molT_sb = consts.tile([C, b_core], F32)
        nc.sync.dma_start(out=molT_sb, in_=io["molT"][:, :])

        w4 = {}  # 4 tiles of [64, 128] per projection weight
        for nm in ("wq", "wmv", "wpk", "wpv"):
            tl = []
            for j in range(4):
                t = consts.tile([64, C], F32, tag=f"{nm}{j}")
                nc.sync.dma_start(out=t, in_=io[nm][j * 64:(j + 1) * 64, :])
                tl.append(t)
            w4[nm] = tl
        wmo_sb = consts.tile([C, A], F32)
        nc.sync.dma_start(out=wmo_sb, in_=io["wmo"][:, :])
        wpo_sb = consts.tile([C, A], F32)
        nc.sync.dma_start(out=wpo_sb, in_=io["wpo"][:, :])

        def bias_ap(k, h):  # k: 0 bq, 1 bmv, 2 bpk, 3 bpv
            j, sl = hof(h)
            return baug[sl, k * 4 + j:k * 4 + j + 1]

        # ---- prologue: fold weights on device ----
        with tc.tile_pool(name="psum_pro", bufs=1, space="PSUM") as ppro:
            wmoT, wpoT = [], []
            for src, dstl, snm in ((wmo_sb, wmoT, "mo"), (wpo_sb, wpoT, "po")):
                for j in range(4):
                    pt = ppro.tile([64, C], F32, tag="pro_t")
                    nc.tensor.transpose(pt, src[:, j * 64:(j + 1) * 64], ident)
                    sb = consts.tile([64, C], F32, tag=f"wT_{snm}_{j}")
                    nc.scalar.copy(out=sb, in_=pt)
                    dstl.append(sb)

            G, Hm, Mp = [], [], []
            for h in range(H):
                j, sl = hof(h)
                for (lh, rh, dst, tag) in (
                    (w4["wq"][j][sl, :], w4["wpk"][j][sl, :], G, "G"),
                    (w4["wmv"][j][sl, :], wpoT[j][sl, :], Hm, "Hm"),
                    (w4["wpv"][j][sl, :], wmoT[j][sl, :], Mp, "Mp"),
                ):
                    pt = ppro.tile([C, C], F32, tag="pro_f")
                    nc.tensor.matmul(pt, lhsT=lh, rhs=rh, start=True, stop=True)
                    sb = consts.tile([C, C], F32, tag=f"{tag}{h}")
                    nc.scalar.copy(out=sb, in_=pt)
                    dst.append(sb)

            # g[:,h] = Wpk_h.T @ bq_h ; u0[:,h] = WpoT_h.T @ bmv_h (+bpo)
            # q2[:,h] = Wq_h.T @ bpk_h
            gp = ppro.tile([C, H], F32, tag="pro_g")
            u0p = ppro.tile([C, H], F32, tag="pro_u0")
            q2p = ppro.tile([C, H], F32, tag="pro_q2")
            for h in range(H):
                j, sl = hof(h)
                nc.tensor.matmul(gp[:, h:h + 1], lhsT=w4["wpk"][j][sl, :],
                                 rhs=bias_ap(0, h), start=True, stop=True)
                nc.tensor.matmul(u0p[:, h:h + 1], lhsT=wpoT[j][sl, :],
                                 rhs=bias_ap(1, h), start=True, stop=True)
                nc.tensor.matmul(q2p[:, h:h + 1], lhsT=w4["wq"][j][sl, :],
                                 rhs=bias_ap(2, h), start=True, stop=True)
            g_sb = consts.tile([C, H], F32)
            nc.scalar.copy(out=g_sb, in_=gp)
            u0_sb = consts.tile([C, H], F32)
            nc.vector.tensor_scalar_add(u0_sb, u0p, bcol[:, 0:1])  # + bpo
            q2_sb = consts.tile([C, H], F32)
            nc.scalar.copy(out=q2_sb, in_=q2p)

            # bias_vec = Wmo @ bpv + bmo (accumulated in its own bank)
            bvp = ppro.tile([C, 1], F32, tag="pro_bv")
            for j in range(4):
                nc.tensor.matmul(bvp, lhsT=wmoT[j], rhs=baug[:, 12 + j:13 + j],
                                 start=(j == 0), stop=(j == 3))
            bv_sb = consts.tile([C, 1], F32)
            nc.vector.tensor_scalar_add(bv_sb, bvp, bcol[:, 1:2])  # + bmo

            # c0col[h] = bq_h . bpk_h
            c0big = consts.tile([1, A], F32)
            nc.vector.tensor_mul(c0big, brow[:, 0:A], brow[:, A:2 * A])
            c0row = consts.tile([1, H], F32)
            nc.vector.reduce_sum(
                c0row, c0big.rearrange("p (h d) -> p h d", h=H),
                axis=mybir.AxisListType.X)
            c0cp = ppro.tile([H, 1], F32, tag="pro_c0")
            nc.tensor.transpose(c0cp, c0row, ident[0:1, 0:1])
            c0col = consts.tile([H, 1], F32)
            nc.scalar.copy(out=c0col, in_=c0cp)

        # ---- pools for the steady state ----
        pn_pool = ctx.enter_context(tc.tile_pool(name="pn", bufs=3))
        pts_pool = ctx.enter_context(tc.tile_pool(name="pts", bufs=3))
        eb_pool = ctx.enter_context(tc.tile_pool(name="eb", bufs=n_grp + 1))
        eT_pool = ctx.enter_context(tc.tile_pool(name="eT", bufs=3))
        og_pool = ctx.enter_context(tc.tile_pool(name="og", bufs=2))
        sm_pool = ctx.enter_context(tc.tile_pool(name="sm", bufs=2))
        avt_pool = ctx.enter_context(tc.tile_pool(name="avt", bufs=2))

        pp_pt = ctx.enter_context(tc.tile_pool(name="pp_pt", bufs=2, space="PSUM"))
        pp_eT = ctx.enter_context(tc.tile_pool(name="pp_eT", bufs=2, space="PSUM"))
        pp_am = ctx.enter_context(tc.tile_pool(name="pp_am", bufs=2, space="PSUM"))
        pp_s1 = ctx.enter_context(tc.tile_pool(name="pp_s1", bufs=1, space="PSUM"))
        pp_pa = ctx.enter_context(tc.tile_pool(name="pp_pa", bufs=1, space="PSUM"))

        # s1 bank: single-shot matmul outputs only.
        #   cols 0:8 kvT | 8:16 UT | 16:24 atT | 24:25 kc col (r0:8)
        #   | 25:33 rmrow (r0) | 128:256 U' (r0:8) | 256:320 scores
        #   | 320:448 avgT (r0:32) + molT out (r0:4, reused at the end)
        #   | 448:512 rm broadcast
        # pa bank: the A accumulator [8,129] plus the (ordered) mf column.
        s1 = pp_s1.tile([C, 512], F32)
        pa = pp_pa.tile([C, 512], F32)
        molcols = consts.tile([C, b_core], F32, tag="molcols")

        prev_mf_inst = None
        for b in range(b_core):
            # ---- per-batch tiny precompute (all single-shot matmuls) ----
            mcol = molT_sb[:, b:b + 1]
            kvp = s1[:, 0:H]
            for h in range(H):
                nc.tensor.matmul(kvp[:, h:h + 1], lhsT=G[h], rhs=mcol,
                                 start=True, stop=True)
            kvT_sb = sm_pool.tile([C, H], F32, tag="kvT")
            nc.vector.tensor_add(kvT_sb, kvp, g_sb)

            utp = s1[:, 8:8 + H]
            for h in range(H):
                nc.tensor.matmul(utp[:, h:h + 1], lhsT=Hm[h], rhs=mcol,
                                 start=True, stop=True)
            ut_sb = sm_pool.tile([C, H], F32, tag="ut")
            nc.vector.tensor_add(ut_sb, utp, u0_sb)

            # kc column and ek = exp(kc)
            kcp = s1[0:H, 24:25]
            nc.tensor.matmul(kcp, lhsT=q2_sb, rhs=mcol, start=True, stop=True)
            kc_sb = sm_pool.tile([H, 1], F32, tag="kc")
            nc.vector.tensor_add(kc_sb, kcp, c0col)
            ek_sb = sm_pool.tile([H, 1], F32, tag="ek")
            nc.scalar.activation(out=ek_sb, in_=kc_sb, func=EXP)

            # U' (heads x channels) scaled by ek, with trailing ek column
            unp = s1[0:H, 128:256]
            nc.tensor.transpose(unp, ut_sb, ident)
            un_sb = sm_pool.tile([H, C + 1], F32, tag="un")
            nc.vector.tensor_scalar_mul(un_sb[:, 0:C], unp, ek_sb)
            nc.vector.tensor_copy(out=un_sb[:, C:C + 1], in_=ek_sb)
            if io["debug"] and b == 0:
                nc.sync.dma_start(out=io["dbg_kvt"][:, :], in_=kvT_sb)
                nc.sync.dma_start(out=io["dbg_un"][:, :], in_=un_sb)

            a_ps = pa[0:H, 0:C + 1]  # A accumulator [8, 129]

            eb_tiles = []
            for g in range(n_grp):
                pn = pn_pool.tile([C, GT, C], F32, tag="pn")
                nc.sync.dma_start(
                    out=pn,
                    in_=prot[b, g * GT * 128:(g + 1) * GT * 128, :]
                    .rearrange("(t x) c -> x t c", x=128))

                sc = s1[:, 256:256 + GT * H]
                for ti in range(GT):
                    ptp = pp_pt.tile([C, C], F32, tag="ptp")
                    nc.tensor.transpose(ptp, pn[:, ti, :], ident)
                    pts = pts_pool.tile([C, C], F32, tag="pts")
                    nc.scalar.copy(out=pts, in_=ptp)
                    nc.tensor.matmul(sc[:, ti * H:(ti + 1) * H], lhsT=pts,
                                     rhs=kvT_sb, start=True, stop=True)

                eb = eb_pool.tile([C, 2 * GT * H], F32, tag="eb")
                if io["debug"] and b == 0 and g == 0:
                    dsc = eb_pool.tile([C, GT * H], F32, tag="dbg_sc")
                    nc.scalar.copy(out=dsc, in_=sc)
                    nc.sync.dma_start(out=io["dbg_sc"][:, :], in_=dsc)
                nc.scalar.activation(out=eb[:, 0:GT * H], in_=sc, func=EXP,
                                     scale=SCALE)
                nc.scalar.activation(out=eb[:, GT * H:], in_=sc, func=EXP)
                eb_tiles.append(eb)
                if io["debug"] and b == 0 and g == 0:
                    nc.sync.dma_start(out=io["dbg_eb"][:, :], in_=eb)

                og = og_pool.tile([C, GT, C], F32, tag="og")
                for ti in range(GT):
                    t = g * GT + ti
                    st, sp = (t == 0), (t == t_tiles - 1)
                    i1 = nc.tensor.matmul(
                        a_ps[:, 0:C], lhsT=eb[:, ti * H:(ti + 1) * H],
                        rhs=pn[:, ti, :], start=st, stop=sp,
                        skip_group_check=True)
                    if st and prev_mf_inst is not None:
                        tile.add_dep_helper(i1.ins, prev_mf_inst,
                                            reason="A accum after prior mf")
                    i2 = nc.tensor.matmul(a_ps[:, C:C + 1],
                                          lhsT=eb[:, ti * H:(ti + 1) * H],
                                          rhs=onescol, start=False, stop=sp,
                                          skip_group_check=True)
                    if st:
                        # i1's start=True clears the whole bank's has_written
                        # bits; the col-128 write must come after it.
                        tile.add_dep_helper(i2.ins, i1.ins,
                                            reason="A col128 after bank clear")

                    # p2m attention -> prot output tile
                    eTp = pp_eT.tile([H, C], F32, tag="eTp")
                    nc.tensor.transpose(
                        eTp, eb[:, GT * H + ti * H:GT * H + (ti + 1) * H], ident)
                    eT = eT_pool.tile([H, C], F32, tag="eT")
                    nc.scalar.copy(out=eT, in_=eTp)
                    am = pp_am.tile([C, C + 1], F32, tag="am")
                    nc.tensor.matmul(am, lhsT=eT, rhs=un_sb, start=True, stop=True)
                    r2 = sm_pool.tile([C, 1], F32, tag="r2")
                    nc.vector.reciprocal(r2, am[:, C:C + 1])
                    nc.vector.scalar_tensor_tensor(
                        out=og[:, ti, :], in0=am[:, 0:C], scalar=r2,
                        in1=pn[:, ti, :], op0=MULT, op1=ADD)
                nc.sync.dma_start(
                    out=out_prot[b, g * GT * 128:(g + 1) * GT * 128, :]
                    .rearrange("(t x) c -> x t c", x=128),
                    in_=og)

            # ---- batch epilogue: m2p normalization, mol output, avg ----
            if io["debug"] and b == 0:
                da = sm_pool.tile([H, C + 1], F32, tag="dbg_a")
                nc.scalar.copy(out=da, in_=a_ps)
                nc.sync.dma_start(out=io["dbg_a"][:, :], in_=da)
            rm = sm_pool.tile([H, 1], F32, tag="rm")
            nc.vector.reciprocal(rm, a_ps[:, C:C + 1])
            at_sb = sm_pool.tile([H, C], F32, tag="at")
            nc.vector.tensor_scalar_mul(at_sb, a_ps[:, 0:C], rm)
            rm8 = sm_pool.tile([H, 1], F32, tag="rm8")
            nc.vector.tensor_scalar_mul(rm8, rm, 1.0 / H)

            atT = s1[:, 16:16 + H]
            nc.tensor.transpose(atT, at_sb, ident[0:H, 0:H])
            atT_sb = sm_pool.tile([C, H], F32, tag="atT")
            nc.scalar.copy(out=atT_sb, in_=atT)

            mf = pa[:, 200:201]
            mf_inst = None
            for h in range(H):
                mf_inst = nc.tensor.matmul(
                    mf, lhsT=Mp[h], rhs=atT_sb[:, h:h + 1],
                    start=(h == 0), stop=(h == H - 1), skip_group_check=True)
            prev_mf_inst = mf_inst.ins
            bvm = sm_pool.tile([C, 1], F32, tag="bvm")
            nc.vector.tensor_add(bvm, bv_sb, mcol)
            nc.vector.scalar_tensor_tensor(
                out=molcols[:, b:b + 1], in0=mf, scalar=1.0, in1=bvm,
                op0=MULT, op1=ADD)

            # avg attention = sum_h em2p * (rm_h/8): broadcast rm8 to all
            # partitions via PE, then multiply+reduce on DVE per group.
            rmrow_p = s1[0:1, 25:25 + H]
            nc.tensor.transpose(rmrow_p, rm8, ident[0:H, 0:H])
            rmrow = sm_pool.tile([1, H], F32, tag="rmrow")
            nc.scalar.copy(out=rmrow, in_=rmrow_p)
            rmrow8 = sm_pool.tile([1, GT * H], F32, tag="rmrow8")
            for r in range(GT):
                nc.vector.tensor_copy(out=rmrow8[:, r * H:(r + 1) * H], in_=rmrow)
            rmb_p = s1[:, 448:448 + GT * H]
            nc.tensor.matmul(rmb_p, lhsT=onesrow, rhs=rmrow8, start=True, stop=True)
            rmb = sm_pool.tile([C, GT * H], F32, tag="rmb")
            nc.scalar.copy(out=rmb, in_=rmb_p)

            avg_sb = avt_pool.tile([C, t_tiles], F32, tag="avg")
            for g in range(n_grp):
                mw = sm_pool.tile([C, GT * H], F32, tag="mw")
                nc.vector.tensor_mul(mw, eb_tiles[g][:, 0:GT * H], rmb)
                nc.vector.reduce_sum(
                    avg_sb[:, g * GT:(g + 1) * GT],
                    mw.rearrange("p (t h) -> p t h", t=GT),
                    axis=mybir.AxisListType.X)

            avT = s1[0:t_tiles, 320:320 + C]
            nc.tensor.transpose(avT, avg_sb, ident)
            avT_sb = avt_pool.tile([t_tiles, C], F32, tag="avT")
            nc.scalar.copy(out=avT_sb, in_=avT)
            nc.sync.dma_start(
                out=out_avg[b, :].rearrange("(t x) -> t x", x=128), in_=avT_sb)

        # final: mol outputs for all batches
        moT = s1[0:b_core, 320:320 + C]
        nc.tensor.transpose(moT, molcols, ident)
        moT_sb = sm_pool.tile([b_core, C], F32, tag="moT")
        nc.scalar.copy(out=moT_sb, in_=moT)
        nc.sync.dma_start(out=out_mol[:, :], in_=moT_sb)


# ---------------------------------------------------------------------------

_PROGRAM_CACHE = {}


def _get_program(b_core, s):
    key = (b_core, s)
    if key not in _PROGRAM_CACHE:
        _PROGRAM_CACHE[key] = build_program(b_core, s)
    return _PROGRAM_CACHE[key]


def make_in_maps(inputs, n_cores, b_core):
    f = np.float32
    mol = np.ascontiguousarray(inputs["mol_features"], dtype=f)
    prot = np.ascontiguousarray(inputs["prot_features"], dtype=f)
    baug = np.zeros((64, 16), f)
    for k, nm in enumerate(("bq", "bmv", "bpk", "bpv")):
        v = np.asarray(inputs[nm], f)
        for j in range(4):
            baug[:, k * 4 + j] = v[j * 64:(j + 1) * 64]
    bcol = np.stack([inputs["bpo"], inputs["bmo"]], axis=1).astype(f)
    brow = np.concatenate([inputs["bq"], inputs["bpk"]])[None, :].astype(f)
    shared = {
        "wq": np.ascontiguousarray(inputs["Wq"], f),
        "wmv": np.ascontiguousarray(inputs["Wmv"], f),
        "wpk": np.ascontiguousarray(inputs["Wpk"], f),
        "wpv": np.ascontiguousarray(inputs["Wpv"], f),
        "wmo": np.ascontiguousarray(inputs["Wmo"], f),
        "wpo": np.ascontiguousarray(inputs["Wpo"], f),
        "baug": baug,
        "bcol": np.ascontiguousarray(bcol),
        "brow": np.ascontiguousarray(brow),
        "ident": np.eye(128, dtype=f),
        "onescol": np.ones((128, 1), f),
        "onesrow": np.ones((1, 128), f),
    }
    in_maps = []
    for i in range(n_cores):
        sl = slice(i * b_core, (i + 1) * b_core)
        m = dict(shared)
        m["prot"] = np.ascontiguousarray(prot[sl])
        m["molT"] = np.ascontiguousarray(mol[sl].T)
        in_maps.append(m)
    return in_maps


def run(inputs, trace=False, **kwargs):
    from concourse.bass_utils import run_bass_kernel_spmd

    b_core = B_FULL // N_CORES
    nc = _get_program(b_core, S_FULL)
    in_maps = make_in_maps(inputs, N_CORES, b_core)
    res = run_bass_kernel_spmd(nc, in_maps, core_ids=list(range(N_CORES)),
                               trace=trace, **kwargs)
    out_mol = np.concatenate([r["out_mol"] for r in res.results], axis=0)
    out_prot = np.concatenate([r["out_prot"] for r in res.results], axis=0)
    out_avg = np.concatenate([r["out_avg"] for r in res.results], axis=0)
    return (out_mol, out_prot, out_avg), res


def kernel(**inputs):
    outs, _ = run(inputs)
    return outs
